# revision 34
# baseline (speedup 1.0000x reference)
"""DebertaV2 disentangled attention block on 8 TRN2 NeuronCores (Bass/Tile).

Head-sharded tensor parallel (2 heads/core), fp8 internals.

Numerics: the block output is dominated by the residual+LayerNorm path
(attention contributes ~1.8% of output norm), so the attention internals run
in fp8e4m3: projections, skew (relative-position) score gathers via DRAM
shear, QK^T, and attn@V. Weights are pre-scaled x32 on host to stay in fp8
normal range; projection copies descale by 1/32 back to natural scale.

Changes vs the first working version (2.49ms recorded; ~0.18-0.20ms measured):
- One coalesced DMA per bulk input (host packs [P, 4, 2, W] chunks); cuts
  ~30 dma_start issues off the startup critical path; hs/reln arrive as
  512-column chunks so projections start before the whole tensor lands.
- relr (reversed rel embeddings) is no longer shipped: pkT is projected
  from reln and column-reversed in the descale write via a negative-stride
  output AP (saves a 2MB load on the startup HBM path).
- The 4 sub-strips of each skew index r (c2p/p2c x 2 heads) share one
  [P, 4, W_WIN] staging tile and ONE shear write + ONE gather read
  (16 dma_starts instead of 64).
- p2c is injected into the score PSUM via identity-lhsT matmuls and exp
  runs straight out of PSUM (drops the bf16 ssum round-trip and ~22us of
  DVE adds).
- Strip (psk, 3 bufs) and score (st, 3 bufs) PSUM tiles get their own
  tags; attn@V reuses the idle projection tag, so no phase is
  bank-starved or FIFO-blocked behind another phase's slots.
- Scores are emitted c-major: the c=0 blocks depend only on c2p strips
  0-3 (+ their own p2c strip), so the PE no longer stalls ~20us at the
  strips->scores boundary waiting for strip 7's gather; attn@V for c=0
  and its DVE reciprocal chain hide under the c=1 score blocks.
- wo/LN/residual inputs load late ON THE HWDGE QUEUES (queued behind the
  strip DMAs) so they cannot contend with the startup window; the Sqrt
  ACT table is pre-warmed; the post-collective unpack is split in two so
  the first output-dense matmuls start after half the data.

Output dense: AllToAll of 16KB normalized-ctx blocks, then each core
computes only its own 128 rows of ctx @ Wo^T + residual + LayerNorm in f32.
"""

import math

import numpy as np

H = 16
D = 64
HID = 1024
N = 1024
K = 1024
EPS = 1e-7
NCORES = 8
HPC = H // NCORES  # heads per core = 2
DPC = HPC * D      # head dims per core = 128
P = 128
W_WIN = 1151       # skew window width (127 + 1024)
WS = 32.0          # host-side weight scale (keeps fp8 weights in normal range)
SCALE_E = 1.0 / math.sqrt(3.0 * D)  # softmax scale, applied inside exp

_CACHE = {}


def _build():
    import concourse.bass as bass
    import concourse.mybir as mybir
    import concourse.tile as tile
    from concourse import bacc
    from concourse.masks import make_identity
    from contextlib import ExitStack

    f32 = mybir.dt.float32
    bf16 = mybir.dt.bfloat16
    f8 = mybir.dt.float8e4
    DR = mybir.MatmulPerfMode.DoubleRow
    Iden = mybir.ActivationFunctionType.Identity
    Exp = mybir.ActivationFunctionType.Exp
    Sqrt = mybir.ActivationFunctionType.Sqrt
    Recip = mybir.ActivationFunctionType.Reciprocal
    ADD = mybir.AluOpType.add
    MUL = mybir.AluOpType.mult
    SUB = mybir.AluOpType.subtract

    nc = bacc.Bacc(None, target_bir_lowering=False, debug=False)
    names = {}

    with tile.TileContext(nc) as tc, ExitStack() as es:
        dio = es.enter_context(tc.tile_pool(name="dram_io", bufs=1, space="DRAM"))
        dwork = es.enter_context(tc.tile_pool(name="dram_work", bufs=1, space="DRAM"))

        def din(nm, shape, dt=f8):
            t = dio.tile(shape, dt, kind="ExternalInput", name=nm, tag=nm)
            names[nm] = t.name
            return t

        # bulk inputs, host-packed as [P, 4, 2, W] (chunk-major free
        # layout); hs/reln arrive as column chunks so the projections can
        # start before the whole tensor lands
        hs_dr = [din(f"hs_dr{i}", (P, 4, 2, 512)) for i in range(2)]
        reln_dr = [din(f"reln_dr{i}", (P, 4, 2, 512)) for i in range(4)]
        wq_dr = din("wq_dr", (P, 4, 2, P))
        wk_dr = din("wk_dr", (P, 4, 2, P))
        wv_dr = din("wv_dr", (P, 4, 2, P))
        wpk_dr = din("wpk_dr", (P, 4, 2, P))
        wpq_dr = din("wpq_dr", (P, 4, 2, P))
        wo_dr = din("wo_dr", (P, 4, 2, HID))
        hs_rows = din("hs_rows", (P, HID), f32)
        bq_s = din("bq_s", (DPC,), f32)
        bk_s = din("bk_s", (DPC,), f32)
        bpk_s = din("bpk_s", (DPC,), f32)
        bpq_s = din("bpq_s", (DPC,), f32)
        bv_s = din("bv_s", (DPC,), f32)
        bo_t = din("bo", (HID,), f32)
        lng_t = din("ln_g", (HID,), f32)
        lnb_t = din("ln_b", (HID,), f32)

        out_t = dio.tile((P, HID), f32, kind="ExternalOutput", name="out", tag="out")
        names["out"] = out_t.name

        a2a_send = dwork.tile((NCORES * P * P,), f8, name="a2a_send", tag="a2a_send")
        a2a_recv = dwork.tile((NCORES * P * P,), f8, name="a2a_recv", tag="a2a_recv")

        # ---- SBUF / PSUM pools -----------------------------------------
        wt = es.enter_context(tc.tile_pool(name="wt", bufs=1))
        work = es.enter_context(tc.tile_pool(name="work", bufs=1))
        ps = es.enter_context(tc.tile_pool(name="ps", bufs=1, space="PSUM"))

        # ---- bulk input loads FIRST, one DMA per tensor -----------------
        # Alternate sync/scalar HWDGE queues to double the issue rate; order
        # defines when the PE can start (q proj needs wq+hs, pos proj needs
        # wpk/relr; wv is not needed until attn@V, wo until the end).
        _eng = [nc.sync, nc.scalar]
        _load_i = [0]

        def load4(src, width, nm):
            t = wt.tile([P, 4, 2, width], f8, name=nm, tag=nm)
            _eng[_load_i[0] % 2].dma_start(t[:], src[:])
            _load_i[0] += 1
            return t

        wq_sb = load4(wq_dr, P, "wq")
        hs_q = [None, None]
        hs_q[0] = load4(hs_dr[0], 512, "hs0")
        wk_sb = load4(wk_dr, P, "wk")
        hs_q[1] = load4(hs_dr[1], 512, "hs1")
        rel_q = [None] * 4
        rel_q[0] = load4(reln_dr[0], 512, "rel0")
        wpk_sb = load4(wpk_dr, P, "wpk")
        rel_q[1] = load4(reln_dr[1], 512, "rel1")
        wv_sb = load4(wv_dr, P, "wv")
        rel_q[2] = load4(reln_dr[2], 512, "rel2")
        wpq_sb = load4(wpq_dr, P, "wpq")
        rel_q[3] = load4(reln_dr[3], 512, "rel3")


        # ---- small persistent inputs (SWDGE queue: off the HWDGE path) --
        ident8 = wt.tile([P, P], f8, name="ident8", tag="ident8")
        make_identity(nc, ident8[:])

        def bias_tile(nm, src, n=DPC):
            t = wt.tile([n, 1], f32, name=nm, tag=nm)
            nc.gpsimd.dma_start(t[:], bass.AP(src[:].tensor, src[:].offset, [[1, n]]))
            return t

        bq_sb = bias_tile("bq_sb", bq_s)
        bk_sb = bias_tile("bk_sb", bk_s)
        bpk_sb = bias_tile("bpk_sb", bpk_s)
        bpq_sb = bias_tile("bpq_sb", bpq_s)

        bv_bc = []
        for h in range(HPC):
            t = wt.tile([P, D], f32, name=f"bv_bc{h}", tag=f"bv_bc{h}")
            nc.gpsimd.dma_start(t[:], bass.AP(bv_s[:].tensor,
                                              bv_s[:].offset + D * h,
                                              [[0, P], [1, D]]))
            bv_bc.append(t)

        def bcast_tile(nm, src):
            t = wt.tile([P, HID], f32, name=nm, tag=nm)
            nc.gpsimd.dma_start(t[:], bass.AP(src[:].tensor, src[:].offset,
                                              [[0, P], [1, HID]]))
            return t

        # ---- projections (fp8 DoubleRow, K=256 per pass) ----------------
        qT = wt.tile([P, N], f8, name="qT", tag="qT")
        kT = wt.tile([P, N], f8, name="kT", tag="kT")
        pkT = wt.tile([P, 2 * K], f8, name="pkT", tag="pkT")
        pqT = wt.tile([P, 2 * K], f8, name="pqT", tag="pqT")

        def project(dst, w_sb, rhs_parts, width, bias, rev=False):
            # rhs_parts: list of [P, 4, 2, 512] column-chunk tiles
            for c0 in range(0, width, 512):
                pp = ps.tile([P, 512], f32, name="pp", tag="pp", bufs=2)
                for c in range(4):
                    nc.tensor.matmul(pp[:], w_sb[:, c, :, :],
                                     rhs_parts[c0 // 512][:, c, :, :],
                                     start=(c == 0), stop=(c == 3),
                                     perf_mode=DR)
                if rev:
                    # write the 512 outputs column-reversed: dst col
                    # (width-1-c0-j) = projection of input row (c0+j)
                    s = dst[:, 0:512]
                    out_ap = bass.AP(
                        s.tensor, s.offset + (width - 1 - c0),
                        [list(s.ap[0]), [-1, 512]])
                else:
                    out_ap = dst[:, c0:c0 + 512]
                nc.scalar.activation(out_ap, pp[:], Iden,
                                     bias=bias[:], scale=1.0 / WS)

        project(qT, wq_sb, hs_q, N, bq_sb)
        project(kT, wk_sb, hs_q, N, bk_sb)
        # pkT needs rel rows in reversed order; project from reln and
        # reverse in the descale write instead of shipping a 2MB relr copy
        project(pkT, wpk_sb, rel_q, 2 * K, bpk_sb, rev=True)

        # ---- v in [j, d] layout with ones columns (DR lhsT layout) ------
        # Emitted here: v needs only hs+Wv, so it fills the PE window while
        # the reln load for the pq projection finishes.
        # va[pair] free layout: [o(2) x 160]; head h at cols 80h..80h+64
        va = []
        for pair in range(4):
            t = wt.tile([P, 2, 160], f8, name=f"va{pair}", tag=f"va{pair}")
            nc.vector.memset(t[:], 1.0)
            va.append(t)
        for jt in range(8):
            pv = ps.tile([P, DPC], f32, name="pv", tag="pp", bufs=2)
            for c in range(4):
                nc.tensor.matmul(
                    pv[:], hs_q[jt // 4][:, c, :, P * (jt % 4):P * (jt % 4 + 1)],
                    wv_sb[:, c, :, :],
                    start=(c == 0), stop=(c == 3), perf_mode=DR)
            for h in range(HPC):
                nc.vector.scalar_tensor_tensor(
                    va[jt // 2][:, jt % 2, 80 * h:80 * h + D],
                    pv[:, D * h:D * (h + 1)], 1.0 / WS, bv_bc[h][:],
                    op0=MUL, op1=ADD)

        project(pqT, wpq_sb, rel_q, 2 * K, bpq_sb)

        def cp_dve(o, i):
            nc.vector.tensor_copy(o, i)

        def cp_act(o, i):
            nc.scalar.activation(o, i, Iden)

        # touch Sqrt once now so its ACT table is resident before the
        # LayerNorm tail (the lazy load costs ~1.3us on the critical path)
        sq_warm = wt.tile([1, 1], f32, name="sq_warm", tag="sq_warm")
        nc.vector.memset(sq_warm[:], 1.0)
        nc.scalar.activation(sq_warm[:], sq_warm[:], Sqrt)

        # ---- skew gathers (via DRAM shear) ------------------------------
        # Per r: 4 sub-strips (c2p h0/h1 from q.pk, p2c h0/h1 from k.pq)
        # share one [P, 4, W_WIN] staging tile, ONE shear write and ONE
        # gather read.  blk[p, k, c] = sub-strip k's [i'=p, m-col c]; gather
        # dst4[p, k, x] = blk[p, k, 127 - p + x].
        # The two heads' K=64 matmuls use disjoint PE row groups (partitions
        # 0-63 vs 64-127), so adjacent issue runs them concurrently.
        dst4 = [None] * 8

        def strip(r):
            blk4 = work.tile([P, 4, W_WIN], f8, name=f"blk{r}", tag="blk",
                             bufs=3)
            kinds = [(qT, pkT, 896 - P * r), (kT, pqT, 897 - P * r)]
            for (c0, w) in ((0, 512), (512, 512), (1024, 127)):
                for kk, (lhsT_src, posT, w0) in enumerate(kinds):
                    pss = []
                    for h in range(HPC):
                        hd = slice(D * h, D * h + D)
                        st = ps.tile([P, 512], f32, name="psk", tag="psk",
                                     bufs=3)
                        nc.tensor.matmul(
                            st[:, 0:w],
                            lhsT_src[hd, P * r:P * (r + 1)],
                            posT[hd, w0 + c0:w0 + c0 + w],
                            start=True, stop=True)
                        pss.append(st)
                    for h in range(HPC):
                        eng = cp_dve if (h + kk + r) % 2 else cp_act
                        eng(blk4[:, 2 * kk + h, c0:c0 + w], pss[h][:, 0:w])
            scr = dwork.tile((P * 4 * W_WIN,), f8, name=f"scr{r}", tag="scr",
                             bufs=4)
            hdr = scr[:].tensor
            nc.sync.dma_start(
                bass.AP(hdr, scr[:].offset,
                        [[W_WIN, P], [P * W_WIN, 4], [1, W_WIN]]),
                blk4[:])
            d4 = wt.tile([P, 4, N], f8, name=f"g{r}", tag=f"g{r}")
            # gather read goes on the OTHER HWDGE queue: its sem-wait on the
            # shear write would otherwise block the next strip's write at
            # the sync sequencer (HWDGE queues are strict FIFO per engine)
            nc.scalar.dma_start(
                d4[:], bass.AP(hdr, scr[:].offset + 127,
                               [[W_WIN - 1, P], [P * W_WIN, 4], [1, N]]))
            dst4[r] = d4

        # c2p[h][r] = dst4[r][:, h, :] ([128 i, 1024 j])
        # p2cg[h][jt] = dst4[jt][:, 2+h, :] ([128 j, 1024 i])

        # ---- scores per (head, j-tile): all terms accumulate in PSUM ----
        # st[j, i-block] = K^T Q (matmul) + c2p^T (transposed-in identity
        # matmuls) + p2c (identity-lhsT matmuls); exp runs on ACT straight
        # out of PSUM into the fp8 DR rhs layout e2.
        e2 = [[wt.tile([P, 2, N], f8, name=f"e2_{h}_{pr}", tag=f"e2_{h}_{pr}")
               for pr in range(4)] for h in range(HPC)]

        def score_block(jt, c):
            # needs c2p strips 4c..4c+3 and p2c strip jt
            sts = []
            for h in range(HPC):
                hd = slice(D * h, D * h + D)
                st = ps.tile([P, 512], f32, name="st", tag="st", bufs=3)
                nc.tensor.matmul(st[:], kT[hd, P * jt:P * (jt + 1)],
                                 qT[hd, 512 * c:512 * (c + 1)],
                                 start=True, stop=False)
                sts.append(st)
            for h in range(HPC):
                for rr in range(4):
                    r = 4 * c + rr
                    nc.tensor.matmul(sts[h][:, P * rr:P * (rr + 1)],
                                     dst4[r][:, h, P * jt:P * (jt + 1)],
                                     ident8[:], start=False, stop=False)
            for h in range(HPC):
                nc.tensor.matmul(sts[h][:],
                                 ident8[:],
                                 dst4[jt][:, 2 + h, 512 * c:512 * (c + 1)],
                                 start=False, stop=True)
            for h in range(HPC):
                nc.scalar.activation(e2[h][jt // 2][:, jt % 2,
                                                    512 * c:512 * (c + 1)],
                                     sts[h][:], Exp, scale=SCALE_E)

        # ---- attn @ v with ones-trick denominators (fp8 DR) -------------
        ctx8 = wt.tile([P, N], f8, name="ctx8", tag="ctx8")

        def attnv_block(c, h):
            pb = ps.tile([65, 512], f32, name="pb", tag="pp", bufs=2)
            for pair in range(4):
                nc.tensor.matmul(pb[:],
                                 va[pair][:, :, 80 * h:80 * h + 65],
                                 e2[h][pair][:, :, 512 * c:512 * (c + 1)],
                                 start=(pair == 0), stop=(pair == 3),
                                 perf_mode=DR)
            rc = work.tile([1, 512], f32, name="rc", tag="rc", bufs=2)
            nc.vector.reciprocal(rc[:], pb[64:65, :])
            rcb = work.tile([D, 512], f32, name="rcb", tag="rcb", bufs=2)
            nc.gpsimd.partition_broadcast(rcb[:], rc[:])
            nc.vector.scalar_tensor_tensor(
                ctx8[D * h:D * (h + 1), 512 * c:512 * (c + 1)],
                pb[0:64, :], WS, rcb[:], op0=MUL, op1=MUL)

        # ---- emission: strips, then scores c-major ----------------------
        # c=0 score blocks need only c2p strips 0-3 (+ own p2c strip), so
        # they fill the PE while strips 4-7's gathers are still in flight;
        # attn@V for c=0 (matmuls + DVE reciprocal chain) hides under the
        # c=1 score blocks.
        for r in range(8):
            strip(r)
        for jt in range(8):
            score_block(jt, 0)
        for h in range(HPC):
            attnv_block(0, h)
        for jt in range(8):
            score_block(jt, 1)
        for h in range(HPC):
            attnv_block(1, h)

        # wo / residual / LN inputs are needed only at the tail; loading
        # them here keeps them off the startup HBM-bandwidth critical path
        wo_sb = load4(wo_dr, HID, "wo")

        def bcast_tile_late(nm, src, eng):
            t = wt.tile([P, HID], f32, name=nm, tag=nm)
            eng.dma_start(t[:], bass.AP(src[:].tensor, src[:].offset,
                                        [[0, P], [1, HID]]))
            return t

        bo_bc = bcast_tile_late("bo_bc", bo_t, nc.scalar)
        g_bc = bcast_tile_late("g_bc", lng_t, nc.sync)
        b_bc = bcast_tile_late("b_bc", lnb_t, nc.scalar)
        hsr_sb = wt.tile([P, HID], f32, name="hsr_sb", tag="hsr_sb")
        nc.sync.dma_start(hsr_sb[:], hs_rows[:])
        hsbo = wt.tile([P, HID], f32, name="hsbo", tag="hsbo")
        nc.vector.tensor_add(hsbo[:], hsr_sb[:], bo_bc[:])

        # ---- AllToAll of normalized ctx blocks --------------------------
        hdr = a2a_send[:].tensor
        nc.sync.dma_start(
            bass.AP(hdr, a2a_send[:].offset, [[P, P], [P * P, NCORES], [1, P]]),
            ctx8[:])
        nc.gpsimd.collective_compute(
            "AllToAll", mybir.AluOpType.bypass,
            replica_groups=[list(range(NCORES))],
            ins=[a2a_send[:]], outs=[a2a_recv[:]])
        ctx_asm = wt.tile([P, NCORES, P], f8, name="ctx_asm", tag="ctx_asm")
        hdr2 = a2a_recv[:].tensor
        nc.sync.dma_start(
            ctx_asm[:, 0:4, :],
            bass.AP(hdr2, a2a_recv[:].offset, [[P, P], [P * P, 4], [1, P]]))
        nc.scalar.dma_start(
            ctx_asm[:, 4:8, :],
            bass.AP(hdr2, a2a_recv[:].offset + 4 * P * P,
                    [[P, P], [P * P, 4], [1, P]]))

        # ---- output dense (own 128 rows) + residual + LayerNorm ---------
        x = wt.tile([P, HID], f32, name="x", tag="x")
        for oc in range(2):
            po = ps.tile([P, 512], f32, name="po", tag="pp", bufs=2)
            for cc in range(4):
                nc.tensor.matmul(po[:], ctx_asm[:, 2 * cc:2 * cc + 2, :],
                                 wo_sb[:, cc, :, 512 * oc:512 * (oc + 1)],
                                 start=(cc == 0), stop=(cc == 3), perf_mode=DR)
            nc.vector.scalar_tensor_tensor(
                x[:, 512 * oc:512 * (oc + 1)], po[:], 1.0 / (WS * WS),
                hsbo[:, 512 * oc:512 * (oc + 1)], op0=MUL, op1=ADD)

        stats = wt.tile([P, 2, 6], f32, name="stats", tag="stats")
        mv = wt.tile([P, 2], f32, name="mv", tag="mv")
        for s in range(2):
            nc.vector.bn_stats(stats[:, s, :], x[:, 512 * s:512 * (s + 1)])
        nc.vector.bn_aggr(mv[:], stats[:])
        epsb = wt.tile([P, 1], f32, name="epsb", tag="epsb")
        nc.vector.memset(epsb[:], EPS)
        std = wt.tile([P, 1], f32, name="std", tag="std")
        nc.scalar.activation(std[:], mv[:, 1:2], Sqrt, bias=epsb[:])
        rstd = wt.tile([P, 1], f32, name="rstd", tag="rstd")
        nc.vector.reciprocal(rstd[:], std[:])

        t1 = wt.tile([P, HID], f32, name="t1", tag="t1")
        yout = wt.tile([P, HID], f32, name="yout", tag="yout")
        for s in range(2):
            cs = slice(512 * s, 512 * (s + 1))
            nc.vector.scalar_tensor_tensor(t1[:, cs], x[:, cs], mv[:, 0:1],
                                           g_bc[:, cs], op0=SUB, op1=MUL)
            nc.vector.scalar_tensor_tensor(yout[:, cs], t1[:, cs], rstd[:],
                                           b_bc[:, cs], op0=MUL, op1=ADD)
            nc.sync.dma_start(out_t[:, cs], yout[:, cs])

    nc.compile()
    return nc, names


def _get_compiled():
    if "nc" not in _CACHE:
        nc, names = _build()
        _CACHE["nc"] = nc
        _CACHE["names"] = names
    return _CACHE["nc"], _CACHE["names"]


def _dr_pack4(mat, width):
    """(HID, width) -> (P, 4, 2, width): chunk-major DR k-tile pairing.

    [p, c, o, x] = mat[c*256 + o*128 + p, x] -- each [p, c] slice is the
    lhsT of one K=256 DoubleRow pass."""
    return np.ascontiguousarray(
        mat.reshape(4, 2, P, width).transpose(2, 0, 1, 3))


def _prep_in_maps(inputs):
    import ml_dtypes

    F8 = ml_dtypes.float8_e4m3
    hs = np.asarray(inputs["hidden_states"], np.float32)[0]      # (N, HID)
    rel = np.asarray(inputs["rel_embeddings"], np.float32)       # (2K, HID)

    hs_p = _dr_pack4(np.ascontiguousarray(hs.T), N).astype(F8)
    rel_p = _dr_pack4(np.ascontiguousarray(rel.T), 2 * K).astype(F8)
    hs_c = [np.ascontiguousarray(hs_p[:, :, :, 512 * i:512 * (i + 1)])
            for i in range(2)]
    rel_c = [np.ascontiguousarray(rel_p[:, :, :, 512 * i:512 * (i + 1)])
             for i in range(4)]
    wo_dr = _dr_pack4(
        WS * np.ascontiguousarray(np.asarray(inputs["Wo"], np.float32).T),
        HID).astype(F8)

    def w_core(w, r):
        w = np.asarray(w, np.float32)
        return _dr_pack4(
            WS * np.ascontiguousarray(w[DPC * r:DPC * (r + 1), :].T), DPC
        ).astype(F8)

    in_maps = []
    for r in range(NCORES):
        m = {
            "hs_dr0": hs_c[0], "hs_dr1": hs_c[1],
            "reln_dr0": rel_c[0], "reln_dr1": rel_c[1],
            "reln_dr2": rel_c[2], "reln_dr3": rel_c[3],
            "wq_dr": w_core(inputs["Wq"], r),
            "wk_dr": w_core(inputs["Wk"], r),
            "wv_dr": w_core(inputs["Wv"], r),
            "wpk_dr": w_core(inputs["Wpk"], r),
            "wpq_dr": w_core(inputs["Wpq"], r),
            "wo_dr": wo_dr,
            "hs_rows": np.ascontiguousarray(hs[P * r:P * (r + 1), :]),
            "bq_s": np.asarray(inputs["bq"], np.float32)[DPC * r:DPC * (r + 1)],
            "bk_s": np.asarray(inputs["bk"], np.float32)[DPC * r:DPC * (r + 1)],
            "bpk_s": np.asarray(inputs["bpk"], np.float32)[DPC * r:DPC * (r + 1)],
            "bpq_s": np.asarray(inputs["bpq"], np.float32)[DPC * r:DPC * (r + 1)],
            "bv_s": np.asarray(inputs["bv"], np.float32)[DPC * r:DPC * (r + 1)],
            "bo": np.asarray(inputs["bo"], np.float32),
            "ln_g": np.asarray(inputs["ln_g"], np.float32),
            "ln_b": np.asarray(inputs["ln_b"], np.float32),
        }
        in_maps.append(m)
    return in_maps


def run(inputs, trace=False):
    from concourse.bass_utils import run_bass_kernel_spmd

    nc, names = _get_compiled()
    logical = _prep_in_maps(inputs)
    in_maps = [{names[k]: v for k, v in m.items()} for m in logical]
    res = run_bass_kernel_spmd(nc, in_maps, list(range(NCORES)), trace=trace)
    outs = [res.results[r][names["out"]].astype(np.float32) for r in range(NCORES)]
    full = np.concatenate(outs, axis=0).reshape(1, N, HID)
    return full, res


def kernel(**inputs) -> np.ndarray:
    full, _ = run(inputs, trace=False)
    return full


# revision 35
# speedup vs baseline: 1.0733x; 1.0733x over previous
"""DebertaV2 disentangled attention block on 8 TRN2 NeuronCores (Bass/Tile).

Head-sharded tensor parallel (2 heads/core), fp8 internals.

Numerics: the block output is dominated by the residual+LayerNorm path
(attention contributes ~1.8% of output norm), so the attention internals run
in fp8e4m3: projections, skew (relative-position) score gathers via DRAM
shear, QK^T, and attn@V. Weights are pre-scaled x32 on host to stay in fp8
normal range; projection copies descale by 1/32 back to natural scale.

Changes vs the first working version (2.49ms recorded; ~0.18-0.20ms measured):
- One coalesced DMA per bulk input (host packs [P, 4, 2, W] chunks); cuts
  ~30 dma_start issues off the startup critical path; hs/reln arrive as
  512-column chunks so projections start before the whole tensor lands.
- relr (reversed rel embeddings) is no longer shipped: pkT is projected
  from reln and column-reversed in the descale write via a negative-stride
  output AP (saves a 2MB load on the startup HBM path).
- The 4 sub-strips of each skew index r (c2p/p2c x 2 heads) share one
  [P, 4, W_WIN] staging tile and ONE shear write + ONE gather read
  (16 dma_starts instead of 64).
- p2c is injected into the score PSUM via identity-lhsT matmuls and exp
  runs straight out of PSUM (drops the bf16 ssum round-trip and ~22us of
  DVE adds).
- Strip (psk, 3 bufs) and score (st, 3 bufs) PSUM tiles get their own
  tags; attn@V reuses the idle projection tag, so no phase is
  bank-starved or FIFO-blocked behind another phase's slots.
- Scores are emitted c-major: the c=0 blocks depend only on c2p strips
  0-3 (+ their own p2c strip), so the PE no longer stalls ~20us at the
  strips->scores boundary waiting for strip 7's gather; attn@V for c=0
  and its DVE reciprocal chain hide under the c=1 score blocks.
- wo/LN/residual inputs load late ON THE HWDGE QUEUES (queued behind the
  strip DMAs) so they cannot contend with the startup window; the Sqrt
  ACT table is pre-warmed; the post-collective unpack is split in two so
  the first output-dense matmuls start after half the data.

Output dense: AllToAll of 16KB normalized-ctx blocks, then each core
computes only its own 128 rows of ctx @ Wo^T + residual + LayerNorm in f32.
"""

import math

import numpy as np

H = 16
D = 64
HID = 1024
N = 1024
K = 1024
EPS = 1e-7
NCORES = 8
HPC = H // NCORES  # heads per core = 2
DPC = HPC * D      # head dims per core = 128
P = 128
W_WIN = 1151       # skew window width (127 + 1024)
WS = 32.0          # host-side weight scale (keeps fp8 weights in normal range)
SCALE_E = 1.0 / math.sqrt(3.0 * D)  # softmax scale, applied inside exp

_CACHE = {}


def _build():
    import concourse.bass as bass
    import concourse.mybir as mybir
    import concourse.tile as tile
    from concourse import bacc
    from concourse.masks import make_identity
    from contextlib import ExitStack

    f32 = mybir.dt.float32
    bf16 = mybir.dt.bfloat16
    f8 = mybir.dt.float8e4
    DR = mybir.MatmulPerfMode.DoubleRow
    Iden = mybir.ActivationFunctionType.Identity
    Exp = mybir.ActivationFunctionType.Exp
    Sqrt = mybir.ActivationFunctionType.Sqrt
    Recip = mybir.ActivationFunctionType.Reciprocal
    ADD = mybir.AluOpType.add
    MUL = mybir.AluOpType.mult
    SUB = mybir.AluOpType.subtract

    nc = bacc.Bacc(None, target_bir_lowering=False, debug=False)
    names = {}

    with tile.TileContext(nc) as tc, ExitStack() as es:
        dio = es.enter_context(tc.tile_pool(name="dram_io", bufs=1, space="DRAM"))
        dwork = es.enter_context(tc.tile_pool(name="dram_work", bufs=1, space="DRAM"))

        def din(nm, shape, dt=f8):
            t = dio.tile(shape, dt, kind="ExternalInput", name=nm, tag=nm)
            names[nm] = t.name
            return t

        # bulk inputs, host-packed as [P, 4, 2, W] (chunk-major free
        # layout); hs/reln arrive as column chunks so the projections can
        # start before the whole tensor lands
        hs_dr = [din(f"hs_dr{i}", (P, 4, 2, 512)) for i in range(2)]
        reln_dr = [din(f"reln_dr{i}", (P, 4, 2, 512)) for i in range(4)]
        wq_dr = din("wq_dr", (P, 4, 2, P))
        wk_dr = din("wk_dr", (P, 4, 2, P))
        wv_dr = din("wv_dr", (P, 4, 2, P))
        wpk_dr = din("wpk_dr", (P, 4, 2, P))
        wpq_dr = din("wpq_dr", (P, 4, 2, P))
        wo_dr = din("wo_dr", (P, 4, 2, HID))
        hs_rows = din("hs_rows", (P, HID), f32)
        bq_s = din("bq_s", (DPC,), f32)
        bk_s = din("bk_s", (DPC,), f32)
        bpk_s = din("bpk_s", (DPC,), f32)
        bpq_s = din("bpq_s", (DPC,), f32)
        bv_s = din("bv_s", (DPC,), f32)
        bo_t = din("bo", (HID,), f32)
        lng_t = din("ln_g", (HID,), f32)
        lnb_t = din("ln_b", (HID,), f32)

        out_t = dio.tile((P, HID), f32, kind="ExternalOutput", name="out", tag="out")
        names["out"] = out_t.name

        a2a_send = dwork.tile((NCORES * P * P,), f8, name="a2a_send", tag="a2a_send")
        a2a_recv = dwork.tile((NCORES * P * P,), f8, name="a2a_recv", tag="a2a_recv")

        # ---- SBUF / PSUM pools -----------------------------------------
        wt = es.enter_context(tc.tile_pool(name="wt", bufs=1))
        work = es.enter_context(tc.tile_pool(name="work", bufs=1))
        ps = es.enter_context(tc.tile_pool(name="ps", bufs=1, space="PSUM"))

        # ---- bulk input loads FIRST, one DMA per tensor -----------------
        # Alternate sync/scalar HWDGE queues to double the issue rate; order
        # defines when the PE can start (q proj needs wq+hs, pos proj needs
        # wpk/relr; wv is not needed until attn@V, wo until the end).
        _eng = [nc.sync, nc.scalar]
        _load_i = [0]

        def load4(src, width, nm):
            t = wt.tile([P, 4, 2, width], f8, name=nm, tag=nm)
            _eng[_load_i[0] % 2].dma_start(t[:], src[:])
            _load_i[0] += 1
            return t

        wq_sb = load4(wq_dr, P, "wq")
        hs_q = [None, None]
        hs_q[0] = load4(hs_dr[0], 512, "hs0")
        wk_sb = load4(wk_dr, P, "wk")
        hs_q[1] = load4(hs_dr[1], 512, "hs1")
        rel_q = [None] * 4
        rel_q[0] = load4(reln_dr[0], 512, "rel0")
        wpk_sb = load4(wpk_dr, P, "wpk")
        rel_q[1] = load4(reln_dr[1], 512, "rel1")
        wv_sb = load4(wv_dr, P, "wv")
        rel_q[2] = load4(reln_dr[2], 512, "rel2")
        wpq_sb = load4(wpq_dr, P, "wpq")
        rel_q[3] = load4(reln_dr[3], 512, "rel3")


        # ---- small persistent inputs (SWDGE queue: off the HWDGE path) --
        ident8 = wt.tile([P, P], f8, name="ident8", tag="ident8")
        make_identity(nc, ident8[:])

        def bias_tile(nm, src, n=DPC):
            t = wt.tile([n, 1], f32, name=nm, tag=nm)
            nc.gpsimd.dma_start(t[:], bass.AP(src[:].tensor, src[:].offset, [[1, n]]))
            return t

        bq_sb = bias_tile("bq_sb", bq_s)
        bk_sb = bias_tile("bk_sb", bk_s)
        bpk_sb = bias_tile("bpk_sb", bpk_s)
        bpq_sb = bias_tile("bpq_sb", bpq_s)

        bv_bc = []
        for h in range(HPC):
            t = wt.tile([P, D], f32, name=f"bv_bc{h}", tag=f"bv_bc{h}")
            nc.gpsimd.dma_start(t[:], bass.AP(bv_s[:].tensor,
                                              bv_s[:].offset + D * h,
                                              [[0, P], [1, D]]))
            bv_bc.append(t)

        def bcast_tile(nm, src):
            t = wt.tile([P, HID], f32, name=nm, tag=nm)
            nc.gpsimd.dma_start(t[:], bass.AP(src[:].tensor, src[:].offset,
                                              [[0, P], [1, HID]]))
            return t

        # ---- projections (fp8 DoubleRow, K=256 per pass) ----------------
        qT = wt.tile([P, N], f8, name="qT", tag="qT")
        kT = wt.tile([P, N], f8, name="kT", tag="kT")
        pkT = wt.tile([P, 2 * K], f8, name="pkT", tag="pkT")
        pqT = wt.tile([P, 2 * K], f8, name="pqT", tag="pqT")

        def project(dst, w_sb, rhs_parts, width, bias, rev=False):
            # rhs_parts: list of [P, 4, 2, 512] column-chunk tiles
            for c0 in range(0, width, 512):
                pp = ps.tile([P, 512], f32, name="pp", tag="pp", bufs=2)
                for c in range(4):
                    nc.tensor.matmul(pp[:], w_sb[:, c, :, :],
                                     rhs_parts[c0 // 512][:, c, :, :],
                                     start=(c == 0), stop=(c == 3),
                                     perf_mode=DR)
                if rev:
                    # write the 512 outputs column-reversed: dst col
                    # (width-1-c0-j) = projection of input row (c0+j)
                    s = dst[:, 0:512]
                    out_ap = bass.AP(
                        s.tensor, s.offset + (width - 1 - c0),
                        [list(s.ap[0]), [-1, 512]])
                else:
                    out_ap = dst[:, c0:c0 + 512]
                nc.scalar.activation(out_ap, pp[:], Iden,
                                     bias=bias[:], scale=1.0 / WS)

        project(qT, wq_sb, hs_q, N, bq_sb)
        project(kT, wk_sb, hs_q, N, bk_sb)
        # pkT needs rel rows in reversed order; project from reln and
        # reverse in the descale write instead of shipping a 2MB relr copy
        project(pkT, wpk_sb, rel_q, 2 * K, bpk_sb, rev=True)

        # ---- v in [j, d] layout with ones columns (DR lhsT layout) ------
        # Emitted here: v needs only hs+Wv, so it fills the PE window while
        # the reln load for the pq projection finishes.
        # va[pair] free layout: [o(2) x 160]; head h at cols 80h..80h+64
        va = []
        for pair in range(4):
            t = wt.tile([P, 2, 160], f8, name=f"va{pair}", tag=f"va{pair}")
            nc.vector.memset(t[:], 1.0)
            va.append(t)
        for jt in range(8):
            pv = ps.tile([P, DPC], f32, name="pv", tag="pp", bufs=2)
            for c in range(4):
                nc.tensor.matmul(
                    pv[:], hs_q[jt // 4][:, c, :, P * (jt % 4):P * (jt % 4 + 1)],
                    wv_sb[:, c, :, :],
                    start=(c == 0), stop=(c == 3), perf_mode=DR)
            for h in range(HPC):
                nc.vector.scalar_tensor_tensor(
                    va[jt // 2][:, jt % 2, 80 * h:80 * h + D],
                    pv[:, D * h:D * (h + 1)], 1.0 / WS, bv_bc[h][:],
                    op0=MUL, op1=ADD)

        project(pqT, wpq_sb, rel_q, 2 * K, bpq_sb)

        def cp_dve(o, i):
            nc.vector.tensor_copy(o, i)

        def cp_act(o, i):
            nc.scalar.activation(o, i, Iden)

        # touch Sqrt once now so its ACT table is resident before the
        # LayerNorm tail (the lazy load costs ~1.3us on the critical path)
        sq_warm = wt.tile([1, 1], f32, name="sq_warm", tag="sq_warm")
        nc.vector.memset(sq_warm[:], 1.0)
        nc.scalar.activation(sq_warm[:], sq_warm[:], Sqrt)

        # ---- skew gathers (via DRAM shear) ------------------------------
        # Per r: 4 sub-strips (c2p h0/h1 from q.pk, p2c h0/h1 from k.pq)
        # share one [P, 4, W_WIN] staging tile, ONE shear write and ONE
        # gather read.  blk[p, k, c] = sub-strip k's [i'=p, m-col c]; gather
        # dst4[p, k, x] = blk[p, k, 127 - p + x].
        # The two heads' K=64 matmuls use disjoint PE row groups (partitions
        # 0-63 vs 64-127), so adjacent issue runs them concurrently.
        dst_c = [None] * 8  # c2p gathers: [P, 2 heads, N], [i, j] layout
        dst_p = [None] * 8  # p2c gathers: [P, 2 heads, N], [j, i] layout

        def strip_kind(r, kk):
            # kk=0: c2p (q . pos_k, needs only pkT); kk=1: p2c (k . pos_q)
            lhsT_src, posT, w0 = ((qT, pkT, 896 - P * r) if kk == 0 else
                                  (kT, pqT, 897 - P * r))
            blk2 = work.tile([P, 2, W_WIN], f8, name=f"blk{kk}_{r}",
                             tag="blk", bufs=3)
            for (c0, w) in ((0, 512), (512, 512), (1024, 127)):
                pss = []
                for h in range(HPC):
                    hd = slice(D * h, D * h + D)
                    st = ps.tile([P, 512], f32, name="psk", tag="psk",
                                 bufs=3)
                    nc.tensor.matmul(
                        st[:, 0:w],
                        lhsT_src[hd, P * r:P * (r + 1)],
                        posT[hd, w0 + c0:w0 + c0 + w],
                        start=True, stop=True)
                    pss.append(st)
                for h in range(HPC):
                    eng = cp_dve if (h + kk + r) % 2 else cp_act
                    eng(blk2[:, h, c0:c0 + w], pss[h][:, 0:w])
            scr = dwork.tile((P * 2 * W_WIN,), f8, name=f"scr{kk}_{r}",
                             tag=f"scr{kk}", bufs=3)
            hdr = scr[:].tensor
            nc.sync.dma_start(
                bass.AP(hdr, scr[:].offset,
                        [[W_WIN, P], [P * W_WIN, 2], [1, W_WIN]]),
                blk2[:])
            d2 = wt.tile([P, 2, N], f8, name=f"g{kk}_{r}", tag=f"g{kk}_{r}")
            # gather read goes on the OTHER HWDGE queue: its sem-wait on the
            # shear write would otherwise block the next strip's write at
            # the sync sequencer (HWDGE queues are strict FIFO per engine)
            nc.scalar.dma_start(
                d2[:], bass.AP(hdr, scr[:].offset + 127,
                               [[W_WIN - 1, P], [P * W_WIN, 2], [1, N]]))
            (dst_c if kk == 0 else dst_p)[r] = d2

        # c2p[h][r] = dst_c[r][:, h, :] ([128 i, 1024 j])
        # p2cg[h][jt] = dst_p[jt][:, h, :] ([128 j, 1024 i])

        # ---- scores per (head, j-tile): all terms accumulate in PSUM ----
        # st[j, i-block] = K^T Q (matmul) + c2p^T (transposed-in identity
        # matmuls) + p2c (identity-lhsT matmuls); exp runs on ACT straight
        # out of PSUM into the fp8 DR rhs layout e2.
        e2 = [[wt.tile([P, 2, N], f8, name=f"e2_{h}_{pr}", tag=f"e2_{h}_{pr}")
               for pr in range(4)] for h in range(HPC)]

        def score_block(jt, c):
            # needs c2p strips 4c..4c+3 and p2c strip jt
            sts = []
            for h in range(HPC):
                hd = slice(D * h, D * h + D)
                st = ps.tile([P, 512], f32, name="st", tag="st", bufs=3)
                nc.tensor.matmul(st[:], kT[hd, P * jt:P * (jt + 1)],
                                 qT[hd, 512 * c:512 * (c + 1)],
                                 start=True, stop=False)
                sts.append(st)
            for h in range(HPC):
                for rr in range(4):
                    r = 4 * c + rr
                    nc.tensor.matmul(sts[h][:, P * rr:P * (rr + 1)],
                                     dst_c[r][:, h, P * jt:P * (jt + 1)],
                                     ident8[:], start=False, stop=False)
            for h in range(HPC):
                nc.tensor.matmul(sts[h][:],
                                 ident8[:],
                                 dst_p[jt][:, h, 512 * c:512 * (c + 1)],
                                 start=False, stop=True)
            for h in range(HPC):
                nc.scalar.activation(e2[h][jt // 2][:, jt % 2,
                                                    512 * c:512 * (c + 1)],
                                     sts[h][:], Exp, scale=SCALE_E)

        # ---- attn @ v with ones-trick denominators (fp8 DR) -------------
        ctx8 = wt.tile([P, N], f8, name="ctx8", tag="ctx8")

        def attnv_block(c, h):
            pb = ps.tile([65, 512], f32, name="pb", tag="pp", bufs=2)
            for pair in range(4):
                nc.tensor.matmul(pb[:],
                                 va[pair][:, :, 80 * h:80 * h + 65],
                                 e2[h][pair][:, :, 512 * c:512 * (c + 1)],
                                 start=(pair == 0), stop=(pair == 3),
                                 perf_mode=DR)
            rc = work.tile([1, 512], f32, name="rc", tag="rc", bufs=2)
            nc.vector.reciprocal(rc[:], pb[64:65, :])
            rcb = work.tile([D, 512], f32, name="rcb", tag="rcb", bufs=2)
            nc.gpsimd.partition_broadcast(rcb[:], rc[:])
            nc.vector.scalar_tensor_tensor(
                ctx8[D * h:D * (h + 1), 512 * c:512 * (c + 1)],
                pb[0:64, :], WS, rcb[:], op0=MUL, op1=MUL)

        # ---- emission: strips, then scores c-major ----------------------
        # c=0 score blocks need only c2p strips 0-3 (+ own p2c strip), so
        # they fill the PE while strips 4-7's gathers are still in flight;
        # attn@V for c=0 (matmuls + DVE reciprocal chain) hides under the
        # c=1 score blocks.
        for r in range(8):
            strip_kind(r, 0)
        for r in range(4):
            strip_kind(r, 1)
        for jt in range(4):
            strip_kind(jt + 4, 1)
            score_block(jt, 0)
        for jt in range(4, 8):
            score_block(jt, 0)
        for h in range(HPC):
            attnv_block(0, h)
        for jt in range(8):
            score_block(jt, 1)
        for h in range(HPC):
            attnv_block(1, h)

        # wo / residual / LN inputs are needed only at the tail; loading
        # them here keeps them off the startup HBM-bandwidth critical path
        wo_sb = load4(wo_dr, HID, "wo")

        def bcast_tile_late(nm, src, eng):
            t = wt.tile([P, HID], f32, name=nm, tag=nm)
            eng.dma_start(t[:], bass.AP(src[:].tensor, src[:].offset,
                                        [[0, P], [1, HID]]))
            return t

        bo_bc = bcast_tile_late("bo_bc", bo_t, nc.scalar)
        g_bc = bcast_tile_late("g_bc", lng_t, nc.sync)
        b_bc = bcast_tile_late("b_bc", lnb_t, nc.scalar)
        hsr_sb = wt.tile([P, HID], f32, name="hsr_sb", tag="hsr_sb")
        nc.sync.dma_start(hsr_sb[:], hs_rows[:])
        hsbo = wt.tile([P, HID], f32, name="hsbo", tag="hsbo")
        nc.vector.tensor_add(hsbo[:], hsr_sb[:], bo_bc[:])

        # ---- AllToAll of normalized ctx blocks --------------------------
        hdr = a2a_send[:].tensor
        nc.sync.dma_start(
            bass.AP(hdr, a2a_send[:].offset, [[P, P], [P * P, NCORES], [1, P]]),
            ctx8[:])
        nc.gpsimd.collective_compute(
            "AllToAll", mybir.AluOpType.bypass,
            replica_groups=[list(range(NCORES))],
            ins=[a2a_send[:]], outs=[a2a_recv[:]])
        ctx_asm = wt.tile([P, NCORES, P], f8, name="ctx_asm", tag="ctx_asm")
        hdr2 = a2a_recv[:].tensor
        nc.sync.dma_start(
            ctx_asm[:, 0:4, :],
            bass.AP(hdr2, a2a_recv[:].offset, [[P, P], [P * P, 4], [1, P]]))
        nc.scalar.dma_start(
            ctx_asm[:, 4:8, :],
            bass.AP(hdr2, a2a_recv[:].offset + 4 * P * P,
                    [[P, P], [P * P, 4], [1, P]]))

        # ---- output dense (own 128 rows) + residual + LayerNorm ---------
        x = wt.tile([P, HID], f32, name="x", tag="x")
        for oc in range(2):
            po = ps.tile([P, 512], f32, name="po", tag="pp", bufs=2)
            for cc in range(4):
                nc.tensor.matmul(po[:], ctx_asm[:, 2 * cc:2 * cc + 2, :],
                                 wo_sb[:, cc, :, 512 * oc:512 * (oc + 1)],
                                 start=(cc == 0), stop=(cc == 3), perf_mode=DR)
            nc.vector.scalar_tensor_tensor(
                x[:, 512 * oc:512 * (oc + 1)], po[:], 1.0 / (WS * WS),
                hsbo[:, 512 * oc:512 * (oc + 1)], op0=MUL, op1=ADD)

        stats = wt.tile([P, 2, 6], f32, name="stats", tag="stats")
        mv = wt.tile([P, 2], f32, name="mv", tag="mv")
        for s in range(2):
            nc.vector.bn_stats(stats[:, s, :], x[:, 512 * s:512 * (s + 1)])
        nc.vector.bn_aggr(mv[:], stats[:])
        epsb = wt.tile([P, 1], f32, name="epsb", tag="epsb")
        nc.vector.memset(epsb[:], EPS)
        std = wt.tile([P, 1], f32, name="std", tag="std")
        nc.scalar.activation(std[:], mv[:, 1:2], Sqrt, bias=epsb[:])
        rstd = wt.tile([P, 1], f32, name="rstd", tag="rstd")
        nc.vector.reciprocal(rstd[:], std[:])

        t1 = wt.tile([P, HID], f32, name="t1", tag="t1")
        yout = wt.tile([P, HID], f32, name="yout", tag="yout")
        for s in range(2):
            cs = slice(512 * s, 512 * (s + 1))
            nc.vector.scalar_tensor_tensor(t1[:, cs], x[:, cs], mv[:, 0:1],
                                           g_bc[:, cs], op0=SUB, op1=MUL)
            nc.vector.scalar_tensor_tensor(yout[:, cs], t1[:, cs], rstd[:],
                                           b_bc[:, cs], op0=MUL, op1=ADD)
            nc.sync.dma_start(out_t[:, cs], yout[:, cs])

    nc.compile()
    return nc, names


def _get_compiled():
    if "nc" not in _CACHE:
        nc, names = _build()
        _CACHE["nc"] = nc
        _CACHE["names"] = names
    return _CACHE["nc"], _CACHE["names"]


def _dr_pack4(mat, width):
    """(HID, width) -> (P, 4, 2, width): chunk-major DR k-tile pairing.

    [p, c, o, x] = mat[c*256 + o*128 + p, x] -- each [p, c] slice is the
    lhsT of one K=256 DoubleRow pass."""
    return np.ascontiguousarray(
        mat.reshape(4, 2, P, width).transpose(2, 0, 1, 3))


def _prep_in_maps(inputs):
    import ml_dtypes

    F8 = ml_dtypes.float8_e4m3
    hs = np.asarray(inputs["hidden_states"], np.float32)[0]      # (N, HID)
    rel = np.asarray(inputs["rel_embeddings"], np.float32)       # (2K, HID)

    hs_p = _dr_pack4(np.ascontiguousarray(hs.T), N).astype(F8)
    rel_p = _dr_pack4(np.ascontiguousarray(rel.T), 2 * K).astype(F8)
    hs_c = [np.ascontiguousarray(hs_p[:, :, :, 512 * i:512 * (i + 1)])
            for i in range(2)]
    rel_c = [np.ascontiguousarray(rel_p[:, :, :, 512 * i:512 * (i + 1)])
             for i in range(4)]
    wo_dr = _dr_pack4(
        WS * np.ascontiguousarray(np.asarray(inputs["Wo"], np.float32).T),
        HID).astype(F8)

    def w_core(w, r):
        w = np.asarray(w, np.float32)
        return _dr_pack4(
            WS * np.ascontiguousarray(w[DPC * r:DPC * (r + 1), :].T), DPC
        ).astype(F8)

    in_maps = []
    for r in range(NCORES):
        m = {
            "hs_dr0": hs_c[0], "hs_dr1": hs_c[1],
            "reln_dr0": rel_c[0], "reln_dr1": rel_c[1],
            "reln_dr2": rel_c[2], "reln_dr3": rel_c[3],
            "wq_dr": w_core(inputs["Wq"], r),
            "wk_dr": w_core(inputs["Wk"], r),
            "wv_dr": w_core(inputs["Wv"], r),
            "wpk_dr": w_core(inputs["Wpk"], r),
            "wpq_dr": w_core(inputs["Wpq"], r),
            "wo_dr": wo_dr,
            "hs_rows": np.ascontiguousarray(hs[P * r:P * (r + 1), :]),
            "bq_s": np.asarray(inputs["bq"], np.float32)[DPC * r:DPC * (r + 1)],
            "bk_s": np.asarray(inputs["bk"], np.float32)[DPC * r:DPC * (r + 1)],
            "bpk_s": np.asarray(inputs["bpk"], np.float32)[DPC * r:DPC * (r + 1)],
            "bpq_s": np.asarray(inputs["bpq"], np.float32)[DPC * r:DPC * (r + 1)],
            "bv_s": np.asarray(inputs["bv"], np.float32)[DPC * r:DPC * (r + 1)],
            "bo": np.asarray(inputs["bo"], np.float32),
            "ln_g": np.asarray(inputs["ln_g"], np.float32),
            "ln_b": np.asarray(inputs["ln_b"], np.float32),
        }
        in_maps.append(m)
    return in_maps


def run(inputs, trace=False):
    from concourse.bass_utils import run_bass_kernel_spmd

    nc, names = _get_compiled()
    logical = _prep_in_maps(inputs)
    in_maps = [{names[k]: v for k, v in m.items()} for m in logical]
    res = run_bass_kernel_spmd(nc, in_maps, list(range(NCORES)), trace=trace)
    outs = [res.results[r][names["out"]].astype(np.float32) for r in range(NCORES)]
    full = np.concatenate(outs, axis=0).reshape(1, N, HID)
    return full, res


def kernel(**inputs) -> np.ndarray:
    full, _ = run(inputs, trace=False)
    return full
